# revision 24
# baseline (speedup 1.0000x reference)
"""3D Haar DWT (2x2x2 blocks, 8 subbands) on 8 Trainium2 NeuronCores.

Input  x: (2, 16, 64, 128, 128) f32.
Output: tuple of 8 subbands, each (2, 16, 32, 64, 64) f32, subband order
LLL,LLH,LHL,LHH,HLL,HLH,HHL,HHH (filters applied to (D,H,W) resp.).

Strategy (pure data parallel, zero cross-core communication):
  - Flatten (B,C) -> 32 independent slabs of (64,128,128); core i takes 4.
  - Per slab: one 4MB DMA into an SBUF tile laid out as
      partitions = (q, d)           q = h parity (2), d = depth (64)
      free       = (hh, hb, w)      hh = h'-half (2), hb = h' sub (32), w (128)
    so that a single constant 128x128 matrix applied on the partition axis by
    the TensorEngine performs BOTH the D-axis and H-axis Haar butterflies:
      out partition m = (alpha, beta, d')   alpha=D band, beta=H band, d' (32)
      M[q*64 + 2d' + p, alpha*64 + beta*32 + d'] = f_alpha[p]*f_beta[q]*s
    (4 nonzeros per column; full 1/(2*sqrt2) scale folded in).
  - The W-axis butterfly (even +/- odd along w) runs on the VectorEngine with
    stride-2 access patterns, PSUM -> SBUF.
  - Each subband's slab result is one contiguous 512KB DMA to HBM.
"""

import numpy as np

_B, _C, _D, _H, _W = 2, 16, 64, 128, 128
_NCORES = 8
_SLABS = _B * _C  # 32
_SLABS_PER_CORE = _SLABS // _NCORES  # 4


def _haar_filters_np():
    # Bit-identical construction to the reference filter bank.
    s = 1.0 / np.sqrt(2.0)
    L = np.array([s, s], dtype=np.float32)
    H = np.array([s, -s], dtype=np.float32)
    bands = [(a, b, c) for a in "LH" for b in "LH" for c in "LH"]
    filt = np.stack(
        [
            (L if a == "L" else H)[:, None, None]
            * (L if b == "L" else H)[None, :, None]
            * (L if c == "L" else H)[None, None, :]
            for (a, b, c) in bands
        ],
        axis=0,
    )  # (8, 2, 2, 2) float32
    return filt


def _haar_matrix():
    """(128,128) f32 for the D-axis butterfly on the partition axis.

    Input partition  = d*2 + hh         (hh = h-half, d = depth 0..63)
    Output partition = a*64 + d'*2 + hh (a = D band, d' = 0..31)
    (d-major order keeps the DMA access patterns' outer dims large, so the
    HWDGE sprays transfers across all 16 SDMA engines.)
    Entry = f_a[p] * s * s  (d = 2d'+p), i.e. the reference filter value
    filt[a*4, p, 0, 0] — the full 1/(2*sqrt2) magnitude is folded here so
    the H/W butterflies on DVE are pure +/- adds."""
    filt = _haar_filters_np()
    M = np.zeros((128, 128), dtype=np.float32)
    for hh in range(2):
        for a in range(2):
            for dp in range(32):
                for p in range(2):
                    M[(2 * dp + p) * 2 + hh, a * 64 + dp * 2 + hh] = filt[
                        a * 4, p, 0, 0
                    ]
    return M


def _build_bass():
    import concourse.mybir as mybir
    import concourse.tile as tile
    from concourse import bacc

    f32 = mybir.dt.float32
    nc = bacc.Bacc("TRN2", target_bir_lowering=False, debug=False)

    x = nc.dram_tensor("x", [_SLABS_PER_CORE, _D, _H, _W], f32, kind="ExternalInput")
    hm = nc.dram_tensor("hm", [128, 128], f32, kind="ExternalInput")
    y = nc.dram_tensor(
        "y", [8, _SLABS_PER_CORE, _D // 2, _H // 2, _W // 2], f32, kind="ExternalOutput"
    )

    # x[t, d, h, w] with h = hh*64 + hb*2 + q.
    # SBUF layout: partitions (d, hh) [p = d*2+hh], free (hb, q, w) -- each
    # partition's free dim walks a CONTIGUOUS 32KB HBM region (one
    # descriptor per partition), and the DRAM-side AP's outer dim is d:64,
    # which the HWDGE deals round-robin across all 16 SDMA engines.
    xr = x[:, :, :, :].rearrange("t d (hh hb q) w -> t d hh hb q w", hh=2, hb=32, q=2)
    # y[s, t, dp, h', w'] with h' = hh*32 + hb; partition order (dp, hh).
    yr = y[:, :, :, :, :].rearrange("s t dp (hh hb) wp -> s t dp hh hb wp", hh=2)

    with tile.TileContext(nc) as tc:
        with (
            tc.tile_pool(name="const", bufs=1) as cpool,
            tc.tile_pool(name="xin", bufs=2) as xpool,
            tc.tile_pool(name="uband", bufs=1) as upool,
            tc.tile_pool(name="outs", bufs=2) as opool,
            tc.tile_pool(name="stage", bufs=2) as spool,
            tc.tile_pool(name="psum", bufs=2, space="PSUM") as ppool,
        ):
            hmt = cpool.tile([128, 128], f32, tag="hm")
            nc.sync.dma_start(out=hmt[:, :], in_=hm[:, :])

            for t in range(_SLABS_PER_CORE):
                # Whole slab: partitions (d, hh), free (hb, q, w) = 8192.
                # Four quarter-DMAs along the free dim (hb ranges) so the
                # first matmul chunk can start as soon as its quarter lands.
                xt = xpool.tile([128, 8192], f32, tag="xt")
                for c in range(4):
                    nc.sync.dma_start(
                        out=xt[:, c * 2048 : (c + 1) * 2048],
                        in_=xr[t, :, :, c * 8 : (c + 1) * 8],
                    )

                # H-band intermediates (post D+H): free (hb, w) = 4096.
                # Written and read only by DVE -> bufs=1 is race-free.
                u = [
                    upool.tile([128, 4096], f32, tag=f"u{b}", name=f"u{b}_{t}")
                    for b in range(2)
                ]
                # Final subband tiles [beta][gamma]: free (hb, w') = 2048.
                o = [
                    [
                        opool.tile(
                            [128, 2048], f32, tag=f"o{b}{g}", name=f"o{b}{g}_{t}"
                        )
                        for g in range(2)
                    ]
                    for b in range(2)
                ]

                for c in range(4):  # 2048-wide chunks: hb in [8c, 8c+8)
                    pt = ppool.tile([128, 2048], f32, tag="pt")
                    for j in range(4):  # N=512 matmuls (fp32 moving-max)
                        lo = c * 2048 + j * 512
                        nc.tensor.matmul(
                            pt[:, j * 512 : (j + 1) * 512],
                            hmt[:, :],
                            xt[:, lo : lo + 512],
                            start=True,
                            stop=True,
                        )
                    # DVE can't read two PSUM operands; stage via ScalarE
                    # (otherwise idle).
                    ct = spool.tile([128, 2048], f32, tag="ct")
                    nc.scalar.copy(ct[:, :], pt[:, :])
                    # H butterfly on DVE: pair q=0/q=1 (stride 256 blocks).
                    cr = ct[:, :].rearrange("m (hb q w) -> m hb q w", hb=8, q=2)
                    ev, od = cr[:, :, 0, :], cr[:, :, 1, :]
                    u0s = u[0][:, c * 1024 : (c + 1) * 1024].rearrange(
                        "m (hb w) -> m hb w", hb=8
                    )
                    u1s = u[1][:, c * 1024 : (c + 1) * 1024].rearrange(
                        "m (hb w) -> m hb w", hb=8
                    )
                    nc.vector.tensor_add(u0s, ev, od)
                    nc.vector.tensor_sub(u1s, ev, od)

                # W butterfly on DVE: even/odd w (stride 2).
                for b in range(2):
                    ur = u[b][:, :].rearrange("m (hb w2 r) -> m hb w2 r", hb=32, r=2)
                    ev, od = ur[:, :, :, 0], ur[:, :, :, 1]
                    o0 = o[b][0][:, :].rearrange("m (hb w2) -> m hb w2", hb=32)
                    o1 = o[b][1][:, :].rearrange("m (hb w2) -> m hb w2", hb=32)
                    nc.vector.tensor_add(o0, ev, od)
                    nc.vector.tensor_sub(o1, ev, od)

                for s in range(8):
                    a, b, g = (s >> 2) & 1, (s >> 1) & 1, s & 1
                    # Partitions a*64 + (dp, hh) interleaved: one contiguous
                    # 64-partition block per subband.  Issue on SP so the ACT
                    # ring stays free to drain PSUM without delay.
                    nc.sync.dma_start(
                        out=yr[s, t],
                        in_=o[b][g][a * 64 : (a + 1) * 64, :],
                    )
    nc.compile()
    return nc


_NC_CACHE = None


def _get_nc():
    global _NC_CACHE
    if _NC_CACHE is None:
        _NC_CACHE = _build_bass()
    return _NC_CACHE


def _run(x, trace=False, **spmd_kwargs):
    from concourse.bass_utils import run_bass_kernel_spmd

    x = np.ascontiguousarray(x, dtype=np.float32)
    xf = x.reshape(_SLABS, _D, _H, _W)
    M = _haar_matrix()
    in_maps = [
        {
            "x": np.ascontiguousarray(
                xf[i * _SLABS_PER_CORE : (i + 1) * _SLABS_PER_CORE]
            ),
            "hm": M,
        }
        for i in range(_NCORES)
    ]
    res = run_bass_kernel_spmd(
        _get_nc(), in_maps, core_ids=list(range(_NCORES)), trace=trace, **spmd_kwargs
    )
    outs = [r["y"] for r in res.results]  # each (8, 4, 32, 64, 64)
    full = np.concatenate(outs, axis=1)  # (8, 32, 32, 64, 64)
    full = full.reshape(8, _B, _C, _D // 2, _H // 2, _W // 2)
    return full, res


def kernel(**inputs):
    full, _ = _run(inputs["x"])
    return tuple(full[i] for i in range(8))


# revision 25
# speedup vs baseline: 1.1111x; 1.1111x over previous
"""3D Haar DWT (2x2x2 blocks, 8 subbands) on 8 Trainium2 NeuronCores.

Input  x: (2, 16, 64, 128, 128) f32.
Output: tuple of 8 subbands, each (2, 16, 32, 64, 64) f32, subband order
LLL,LLH,LHL,LHH,HLL,HLH,HHL,HHH (filters applied to (D,H,W) resp.).

Strategy (pure data parallel, zero cross-core communication):
  - Flatten (B,C) -> 32 independent slabs of (64,128,128); core i takes 4.
  - Per slab: one 4MB DMA into an SBUF tile laid out as
      partitions = (q, d)           q = h parity (2), d = depth (64)
      free       = (hh, hb, w)      hh = h'-half (2), hb = h' sub (32), w (128)
    so that a single constant 128x128 matrix applied on the partition axis by
    the TensorEngine performs BOTH the D-axis and H-axis Haar butterflies:
      out partition m = (alpha, beta, d')   alpha=D band, beta=H band, d' (32)
      M[q*64 + 2d' + p, alpha*64 + beta*32 + d'] = f_alpha[p]*f_beta[q]*s
    (4 nonzeros per column; full 1/(2*sqrt2) scale folded in).
  - The W-axis butterfly (even +/- odd along w) runs on the VectorEngine with
    stride-2 access patterns, PSUM -> SBUF.
  - Each subband's slab result is one contiguous 512KB DMA to HBM.
"""

import numpy as np

_B, _C, _D, _H, _W = 2, 16, 64, 128, 128
_NCORES = 8
_SLABS = _B * _C  # 32
_SLABS_PER_CORE = _SLABS // _NCORES  # 4


def _haar_filters_np():
    # Bit-identical construction to the reference filter bank.
    s = 1.0 / np.sqrt(2.0)
    L = np.array([s, s], dtype=np.float32)
    H = np.array([s, -s], dtype=np.float32)
    bands = [(a, b, c) for a in "LH" for b in "LH" for c in "LH"]
    filt = np.stack(
        [
            (L if a == "L" else H)[:, None, None]
            * (L if b == "L" else H)[None, :, None]
            * (L if c == "L" else H)[None, None, :]
            for (a, b, c) in bands
        ],
        axis=0,
    )  # (8, 2, 2, 2) float32
    return filt


def _haar_matrix():
    """(128,128) f32 for the D-axis butterfly on the partition axis.

    Input partition  = d*2 + hh         (hh = h-half, d = depth 0..63)
    Output partition = a*64 + d'*2 + hh (a = D band, d' = 0..31)
    (d-major order keeps the DMA access patterns' outer dims large, so the
    HWDGE sprays transfers across all 16 SDMA engines.)
    Entry = f_a[p] * s * s  (d = 2d'+p), i.e. the reference filter value
    filt[a*4, p, 0, 0] — the full 1/(2*sqrt2) magnitude is folded here so
    the H/W butterflies on DVE are pure +/- adds."""
    filt = _haar_filters_np()
    M = np.zeros((128, 128), dtype=np.float32)
    for hh in range(2):
        for a in range(2):
            for dp in range(32):
                for p in range(2):
                    M[(2 * dp + p) * 2 + hh, a * 64 + dp * 2 + hh] = filt[
                        a * 4, p, 0, 0
                    ]
    return M


def _build_bass():
    import concourse.mybir as mybir
    import concourse.tile as tile
    from concourse import bacc

    f32 = mybir.dt.float32
    nc = bacc.Bacc("TRN2", target_bir_lowering=False, debug=False)

    x = nc.dram_tensor("x", [_SLABS_PER_CORE, _D, _H, _W], f32, kind="ExternalInput")
    hm = nc.dram_tensor("hm", [128, 128], f32, kind="ExternalInput")
    y = nc.dram_tensor(
        "y", [8, _SLABS_PER_CORE, _D // 2, _H // 2, _W // 2], f32, kind="ExternalOutput"
    )

    # x[t, d, h, w] with h = hh*64 + hb*2 + q.
    # SBUF layout: partitions (d, hh) [p = d*2+hh], free (hb, q, w) -- each
    # partition's free dim walks a CONTIGUOUS 32KB HBM region (one
    # descriptor per partition), and the DRAM-side AP's outer dim is d:64,
    # which the HWDGE deals round-robin across all 16 SDMA engines.
    xr = x[:, :, :, :].rearrange("t d (hh hb q) w -> t d hh hb q w", hh=2, hb=32, q=2)
    # y[s, t, dp, h', w'] with h' = hh*32 + hb; partition order (dp, hh).
    yr = y[:, :, :, :, :].rearrange("s t dp (hh hb) wp -> s t dp hh hb wp", hh=2)

    with tile.TileContext(nc) as tc:
        with (
            tc.tile_pool(name="const", bufs=1) as cpool,
            tc.tile_pool(name="xin", bufs=2) as xpool,
            tc.tile_pool(name="uband", bufs=1) as upool,
            tc.tile_pool(name="outs", bufs=2) as opool,
            tc.tile_pool(name="stage", bufs=2) as spool,
            tc.tile_pool(name="psum", bufs=2, space="PSUM") as ppool,
        ):
            hmt = cpool.tile([128, 128], f32, tag="hm")
            nc.sync.dma_start(out=hmt[:, :], in_=hm[:, :])

            for t in range(_SLABS_PER_CORE):
                # Whole slab: partitions (d, hh), free (hb, q, w) = 8192.
                # Four quarter-DMAs along the free dim (hb ranges) so the
                # first matmul chunk can start as soon as its quarter lands.
                # Issue inputs on the ACT HWDGE ring: the SP ring carries the
                # output DMAs, and ring FIFO order would make next-slab
                # inputs queue behind them (observed 20us PE stalls).
                xt = xpool.tile([128, 8192], f32, tag="xt")
                for c in range(4):
                    nc.scalar.dma_start(
                        out=xt[:, c * 2048 : (c + 1) * 2048],
                        in_=xr[t, :, :, c * 8 : (c + 1) * 8],
                    )

                # H-band intermediates (post D+H): free (hb, w) = 4096.
                # Written and read only by DVE -> bufs=1 is race-free.
                u = [
                    upool.tile([128, 4096], f32, tag=f"u{b}", name=f"u{b}_{t}")
                    for b in range(2)
                ]
                # Final subband tiles [beta][gamma]: free (hb, w') = 2048.
                o = [
                    [
                        opool.tile(
                            [128, 2048], f32, tag=f"o{b}{g}", name=f"o{b}{g}_{t}"
                        )
                        for g in range(2)
                    ]
                    for b in range(2)
                ]

                for c in range(4):  # 2048-wide chunks: hb in [8c, 8c+8)
                    pt = ppool.tile([128, 2048], f32, tag="pt")
                    for j in range(4):  # N=512 matmuls (fp32 moving-max)
                        lo = c * 2048 + j * 512
                        nc.tensor.matmul(
                            pt[:, j * 512 : (j + 1) * 512],
                            hmt[:, :],
                            xt[:, lo : lo + 512],
                            start=True,
                            stop=True,
                        )
                    # DVE can't read two PSUM operands; stage via ScalarE
                    # (otherwise idle).
                    ct = spool.tile([128, 2048], f32, tag="ct")
                    nc.scalar.copy(ct[:, :], pt[:, :])
                    # H butterfly on DVE: pair q=0/q=1 (stride 256 blocks).
                    cr = ct[:, :].rearrange("m (hb q w) -> m hb q w", hb=8, q=2)
                    ev, od = cr[:, :, 0, :], cr[:, :, 1, :]
                    u0s = u[0][:, c * 1024 : (c + 1) * 1024].rearrange(
                        "m (hb w) -> m hb w", hb=8
                    )
                    u1s = u[1][:, c * 1024 : (c + 1) * 1024].rearrange(
                        "m (hb w) -> m hb w", hb=8
                    )
                    nc.vector.tensor_add(u0s, ev, od)
                    nc.vector.tensor_sub(u1s, ev, od)

                # W butterfly on DVE: even/odd w (stride 2).
                for b in range(2):
                    ur = u[b][:, :].rearrange("m (hb w2 r) -> m hb w2 r", hb=32, r=2)
                    ev, od = ur[:, :, :, 0], ur[:, :, :, 1]
                    o0 = o[b][0][:, :].rearrange("m (hb w2) -> m hb w2", hb=32)
                    o1 = o[b][1][:, :].rearrange("m (hb w2) -> m hb w2", hb=32)
                    nc.vector.tensor_add(o0, ev, od)
                    nc.vector.tensor_sub(o1, ev, od)

                for s in range(8):
                    a, b, g = (s >> 2) & 1, (s >> 1) & 1, s & 1
                    # Partitions a*64 + (dp, hh) interleaved: one contiguous
                    # 64-partition block per subband.  Issue on SP so the ACT
                    # ring stays free to drain PSUM without delay.
                    nc.sync.dma_start(
                        out=yr[s, t],
                        in_=o[b][g][a * 64 : (a + 1) * 64, :],
                    )
    nc.compile()
    return nc


_NC_CACHE = None


def _get_nc():
    global _NC_CACHE
    if _NC_CACHE is None:
        _NC_CACHE = _build_bass()
    return _NC_CACHE


def _run(x, trace=False, **spmd_kwargs):
    from concourse.bass_utils import run_bass_kernel_spmd

    x = np.ascontiguousarray(x, dtype=np.float32)
    xf = x.reshape(_SLABS, _D, _H, _W)
    M = _haar_matrix()
    in_maps = [
        {
            "x": np.ascontiguousarray(
                xf[i * _SLABS_PER_CORE : (i + 1) * _SLABS_PER_CORE]
            ),
            "hm": M,
        }
        for i in range(_NCORES)
    ]
    res = run_bass_kernel_spmd(
        _get_nc(), in_maps, core_ids=list(range(_NCORES)), trace=trace, **spmd_kwargs
    )
    outs = [r["y"] for r in res.results]  # each (8, 4, 32, 64, 64)
    full = np.concatenate(outs, axis=1)  # (8, 32, 32, 64, 64)
    full = full.reshape(8, _B, _C, _D // 2, _H // 2, _W // 2)
    return full, res


def kernel(**inputs):
    full, _ = _run(inputs["x"])
    return tuple(full[i] for i in range(8))
